# revision 27
# baseline (speedup 1.0000x reference)
"""Trainium2 Bass kernel for PhaseCoherenceComputer.

coherence[b,h,q,k] = mean_d cos(phases_q[b,h,q,d] - phases_k[b,h,k,d])
                   = (cos_q @ cos_k^T + sin_q @ sin_k^T) / 64

Shapes: phases_q/k [2, 8, 2048, 64] f32 -> out [2, 8, 2048, 2048] f32.

Strategy (8 NeuronCores, data-parallel over the 16 (b,h) pairs, 2 per core):
- f16 everywhere off-chip (harness tolerance is 2e-2, f16 adds ~2e-4):
  per core 16.8 MB out + 1.5 MB in vs 33.5 MB + 2 MB for the f32
  baseline. The kernel is HBM-write-bound at ~358 GB/s/core, so bytes =
  time; everything else is pipelined under the write stream.
- Pair 0 (ramp-critical): the host ships READY-TO-MATMUL operands
  U = [cos_q^T; sin_q^T], V = [cos_k^T; sin_k^T] as f16 [128, S] blocks,
  chunked so the first matmul fires as soon as the first two 128 KB
  chunks land (~9.7 us) — no on-device trig on the ramp critical path,
  and no ACT Sin-table load gating the start of the output stream.
- Pair 1: host ships range-reduced phases r in [-pi,pi] as f16 [64, S]
  (0.5 MB). Mid-stream, a DVE sign-bit clear builds |r| in partitions
  0:64 and one Sin activation per half with per-partition (scale, bias)
  = (-1, pi/2)/(+1, 0) yields [cos^T; sin^T] (arguments inside the
  accurate [-pi/2, pi/2] spline range). This prep rides in pair-0's
  q-loop slack; a dummy 1-column sin preloads the ACT tables during the
  ramp so no mid-stream table stall occurs.
- One K=128 f16 matmul per [128 q x 512 k] output block. PSUM is carved
  into four [128, 1024] half-tiles (2 banks each): per q-tile, psA
  holds k-blocks 0-1 and psB k-blocks 2-3, so VectorE (psA) and ACT
  (psB) recycle PSUM independently; the PSUM chain (matmul + one
  half-evac ~2.4 us per 2 tiles) stays under the DMA drain period.
  Evacuation applies the 1/64 scale and converts to f16.
- Output DMA: 2 q-tiles per [128, 2*S] f16 SBUF block, one 1 MB
  sync-ring (HWDGE) DMA with 8 KB contiguous per-partition descriptors
  (DRAM layout [8 blocks, 128, 2*S] per pair; host unpermutes). All
  output DMAs ride the SP ring so ACT compute never delays an issue;
  ot bufs=8 lets evacuation run well ahead of the drain. The first and
  last blocks stream as 4 x 256 KB quarter-DMAs fired per half-evac,
  starting the HBM write stream earlier and shrinking the final drain.
- All input DMAs are queued on the sync ring ahead of the output blocks
  (pair-0 chunks first, then pair-1), so the wire is never given
  non-critical bytes while ramp-critical ones wait, and input drains
  during the window where output isn't ready yet.
"""

import sys

import numpy as np

try:
    import concourse.bacc as bacc
except ImportError:  # fresh interpreter without the axon site path
    for _p in ("/opt/trn_rl_repo", "/root/.axon_site/_ro/trn_rl_repo"):
        if _p not in sys.path:
            sys.path.insert(0, _p)
    import concourse.bacc as bacc

import concourse.mybir as mybir
import concourse.tile as tile
from concourse.bass_utils import run_bass_kernel_spmd

F32 = mybir.dt.float32
F16 = mybir.dt.float16
U16 = mybir.dt.uint16
UV_DT = F16  # matmul operand dtype
OUT_DT = F16  # device-side output dtype (host upcasts to f32)

B, H, S, D = 2, 8, 2048, 64
N_CORES = 8
PAIRS_PER_CORE = (B * H) // N_CORES  # 2
Q_TILE = 128  # output rows per matmul (PSUM partitions)
K_TILE = 512  # output cols per matmul
N_QT = S // Q_TILE  # 16
BLK = 2  # q-tiles per output DMA block (1 MB f16)
N_BLK = N_QT // BLK  # 8
HC = S // 2  # half-row chunk for pair-1 sin
EC = 2 * K_TILE  # evac chunk (one PSUM half-tile)

_NC_CACHE = {}


def build_kernel():
    """Per-core SPMD program. pin0 [2, 128, S] f16: pair-0 ready
    cos/sin operand blocks (tensor 0 = V from k-phases, 1 = U from
    q-phases). pin1 [2, 64, S] f16: pair-1 range-reduced phases.
    Output out [PAIRS, N_BLK, 128, BLK*S] f16: block j holds q-tiles
    BLK*j..BLK*j+BLK-1 side by side."""
    nc = bacc.Bacc("TRN2", target_bir_lowering=False, debug=False)
    pin0 = nc.dram_tensor("pin0", [2, 128, S], F16, kind="ExternalInput")
    pin1 = nc.dram_tensor("pin1", [2, 64, S], F16, kind="ExternalInput")
    out = nc.dram_tensor(
        "out", [PAIRS_PER_CORE, N_BLK, 128, BLK * S], OUT_DT, kind="ExternalOutput"
    )
    SIN = mybir.ActivationFunctionType.Sin

    with tile.TileContext(nc) as tc:
        with (
            tc.tile_pool(name="const", bufs=1) as cpool,
            tc.tile_pool(name="raw", bufs=1) as rawpool,
            tc.tile_pool(name="uv", bufs=2) as uvpool,
            tc.tile_pool(name="ot", bufs=8) as opool,
            tc.tile_pool(name="psum", bufs=2, space="PSUM") as ppool,
        ):
            # Per-partition Sin affine for pair 1: top half cos via
            # sin(pi/2 - |r|), bottom half sin via sin(r).
            bias = cpool.tile([128, 1], F32)
            scale = cpool.tile([128, 1], F32)
            tabw = cpool.tile([128, 1], F32)
            nc.vector.memset(bias[0:64, :], np.pi / 2)
            nc.vector.memset(bias[64:128, :], 0.0)
            nc.vector.memset(scale[0:64, :], -1.0)
            nc.vector.memset(scale[64:128, :], 1.0)

            raw1 = (
                rawpool.tile([128, S], F16, tag="vraw", name="vraw"),
                rawpool.tile([128, S], F16, tag="uraw", name="uraw"),
            )
            uvs = {}
            for p in range(PAIRS_PER_CORE):
                uvs[p] = (
                    uvpool.tile([128, S], UV_DT, tag="v", name="v"),
                    uvpool.tile([128, S], UV_DT, tag="u", name="u"),
                )

            # Ramp inputs split across BOTH HWDGE rings so u and v land in
            # parallel (~9.3 us) instead of serially: u + pair-1 on sync,
            # v chunks on scalar. This is only safe because pair-0 needs
            # no on-device trig — ACT has nothing ramp-critical before its
            # first evacuation at ~11.5 us, so its table load can sit
            # behind the v-chunk DMA issues.
            K2 = 2 * K_TILE
            nc.sync.dma_start(out=uvs[0][1][:, 0:K_TILE], in_=pin0[1, :, 0:K_TILE])
            nc.scalar.dma_start(out=uvs[0][0][:, 0:K_TILE], in_=pin0[0, :, 0:K_TILE])
            nc.scalar.dma_start(out=uvs[0][0][:, K_TILE:K2], in_=pin0[0, :, K_TILE:K2])
            nc.scalar.dma_start(out=uvs[0][0][:, K2:S], in_=pin0[0, :, K2:S])
            nc.sync.dma_start(out=uvs[0][1][:, K_TILE:S], in_=pin0[1, :, K_TILE:S])
            nc.sync.dma_start(out=raw1[0][64:128, :], in_=pin1[0])
            nc.sync.dma_start(out=raw1[1][64:128, :], in_=pin1[1])

            # Dummy 1-column sin: pulls the ACT Sin-table loads into the
            # ramp (ACT is otherwise idle there) so pair-1's mid-stream
            # sins don't stall on a ~2.6 us table load.
            nc.scalar.activation(tabw[:], bias[:], SIN)

            # Engine wake-ups: the first PSUM evacuation on a cold engine
            # starts ~0.9 us after its matmul's semaphore fires (vs
            # ~0.1-0.3 us warm). A 1-column op gated on the first input
            # chunk's DMA leaves each engine freshly active right before
            # its first real evacuation, shaving the cold-start gap off
            # the first-output-byte chain.
            wakev = cpool.tile([128, 1], F32)
            wakes = cpool.tile([128, 1], F32)
            nc.vector.tensor_scalar_mul(wakev[:], uvs[0][1][:, 0:1], 1.0)
            nc.scalar.mul(wakes[:], uvs[0][0][:, 0:1], 1.0)

            def abs1(t):
                nc.vector.tensor_scalar(
                    raw1[t][0:64, :].bitcast(U16),
                    raw1[t][64:128, :].bitcast(U16),
                    0x7FFF,
                    None,
                    mybir.AluOpType.bitwise_and,
                )

            def sin1(t, h):
                hs = slice(h * HC, (h + 1) * HC)
                nc.scalar.activation(
                    uvs[1][t][:, hs], raw1[t][:, hs], SIN,
                    bias=bias[:], scale=scale[:],
                )

            def q_tile(p, q, ot, col0, dma_quarters, fine=False):
                v, u = uvs[p][0], uvs[p][1]
                us = u[:, q * Q_TILE : (q + 1) * Q_TILE]
                psA = ppool.tile([128, EC], F32, tag="psA", name="psA")
                psB = ppool.tile([128, EC], F32, tag="psB", name="psB")
                for k in range(2):
                    nc.tensor.matmul(
                        psA[:, k * K_TILE : (k + 1) * K_TILE],
                        us,
                        v[:, k * K_TILE : (k + 1) * K_TILE],
                        start=True,
                        stop=True,
                    )
                for k in range(2):
                    nc.tensor.matmul(
                        psB[:, k * K_TILE : (k + 1) * K_TILE],
                        us,
                        v[:, (k + 2) * K_TILE : (k + 3) * K_TILE],
                        start=True,
                        stop=True,
                    )
                if fine:
                    # First tile of the stream: 512-col evac + 128 KB DMA
                    # chunks, each gated only on its own matmul, so the
                    # first output bytes hit the wire right after matmul
                    # k0 instead of after the whole psA half-tile.
                    for k in range(2):
                        ks = slice(col0 + k * K_TILE, col0 + (k + 1) * K_TILE)
                        nc.vector.tensor_scalar_mul(
                            ot[:, ks], psA[:, k * K_TILE : (k + 1) * K_TILE], 1.0 / D
                        )
                        nc.sync.dma_start(out=dma_quarters[ks], in_=ot[:, ks])
                    for k in range(2):
                        ks = slice(col0 + EC + k * K_TILE, col0 + EC + (k + 1) * K_TILE)
                        nc.scalar.mul(
                            ot[:, ks], psB[:, k * K_TILE : (k + 1) * K_TILE], 1.0 / D
                        )
                        nc.sync.dma_start(out=dma_quarters[ks], in_=ot[:, ks])
                    return
                nc.vector.tensor_scalar_mul(ot[:, col0 : col0 + EC], psA[:], 1.0 / D)
                if dma_quarters is not None:
                    nc.sync.dma_start(
                        out=dma_quarters[col0 : col0 + EC], in_=ot[:, col0 : col0 + EC]
                    )
                nc.scalar.mul(ot[:, col0 + EC : col0 + 2 * EC], psB[:], 1.0 / D)
                if dma_quarters is not None:
                    nc.sync.dma_start(
                        out=dma_quarters[col0 + EC : col0 + 2 * EC],
                        in_=ot[:, col0 + EC : col0 + 2 * EC],
                    )

            # Pair-1 prep spread through pair-0's q-loop: abs once the
            # input DMAs have landed, sins one per block, all done before
            # pair-0's last block.
            prep1 = {
                5: lambda: abs1(0),
                7: lambda: abs1(1),
                8: lambda: sin1(0, 0),
                10: lambda: sin1(0, 1),
                12: lambda: sin1(1, 0),
                14: lambda: sin1(1, 1),
            }

            for p in range(PAIRS_PER_CORE):
                for blk in range(N_BLK):
                    ot = opool.tile([128, BLK * S], OUT_DT, tag="ot", name="ot")
                    split = (p == 0 and blk == 0) or (
                        p == PAIRS_PER_CORE - 1 and blk == N_BLK - 1
                    )
                    dq = None
                    if split:
                        class _Q:  # column-sliced DMA target for this block
                            def __getitem__(_s, cols):
                                return out[p, blk, :, cols]
                        dq = _Q()
                    for j in range(BLK):
                        q = blk * BLK + j
                        q_tile(p, q, ot, j * S, dq, fine=(p == 0 and q == 0))
                        if p == 0 and q in prep1:
                            prep1[q]()
                    if not split:
                        nc.sync.dma_start(out=out[p, blk], in_=ot[:])
    nc.compile()
    return nc


def _prep(ph):
    """[16, S, D] phases -> [16, 64, S] f64 range-reduced transposed."""
    pht = ph.astype(np.float64).transpose(0, 2, 1)  # [16, D, S]
    return np.mod(pht + np.pi, 2 * np.pi) - np.pi


def kernel(phases_q, phases_k, _trace=False):
    pq = np.asarray(phases_q, dtype=np.float32).reshape(B * H, S, D)
    pk = np.asarray(phases_k, dtype=np.float32).reshape(B * H, S, D)
    qr = _prep(pq)  # [16, 64, S] f64
    kr = _prep(pk)

    in_maps = []
    for c in range(N_CORES):
        p0, p1 = 2 * c, 2 * c + 1
        pin0 = np.empty((2, 2 * D, S), dtype=np.float16)  # ready cos/sin
        for t, r in ((0, kr[p0]), (1, qr[p0])):
            pin0[t, :D] = np.cos(r)
            pin0[t, D:] = np.sin(r)
        pin1 = np.stack([kr[p1], qr[p1]]).astype(np.float16)  # [2, 64, S]
        in_maps.append(
            {"pin0": np.ascontiguousarray(pin0), "pin1": np.ascontiguousarray(pin1)}
        )

    if "nc" not in _NC_CACHE:
        _NC_CACHE["nc"] = build_kernel()
    nc = _NC_CACHE["nc"]

    res = run_bass_kernel_spmd(
        nc, in_maps, core_ids=list(range(N_CORES)), trace=_trace
    )
    # [16, N_BLK, 128, BLK*S] -> [16, S, S]: block j holds q-tiles
    # (BLK*j+i) in column slices i*S:(i+1)*S.
    full = np.concatenate([r["out"] for r in res.results], axis=0)
    full = full.reshape(B * H, N_BLK, Q_TILE, BLK, S)
    full = full.transpose(0, 1, 3, 2, 4).reshape(B * H, S, S)
    out = full.astype(np.float32).reshape(B, H, S, S)
    if _trace:
        return out, res
    return out


# revision 28
# speedup vs baseline: 1.0780x; 1.0780x over previous
"""Trainium2 Bass kernel for PhaseCoherenceComputer.

coherence[b,h,q,k] = mean_d cos(phases_q[b,h,q,d] - phases_k[b,h,k,d])
                   = (cos_q @ cos_k^T + sin_q @ sin_k^T) / 64

Shapes: phases_q/k [2, 8, 2048, 64] f32 -> out [2, 8, 2048, 2048] f32.

Strategy (8 NeuronCores, data-parallel over the 16 (b,h) pairs, 2 per core):
- f16 everywhere off-chip (harness tolerance is 2e-2, f16 adds ~2e-4):
  per core 16.8 MB out + 1.5 MB in vs 33.5 MB + 2 MB for the f32
  baseline. The kernel is HBM-write-bound at ~358 GB/s/core, so bytes =
  time; everything else is pipelined under the write stream.
- Pair 0 (ramp-critical): the host ships READY-TO-MATMUL operands
  U = [cos_q^T; sin_q^T], V = [cos_k^T; sin_k^T] as f16 [128, S] blocks,
  chunked so the first matmul fires as soon as the first two 128 KB
  chunks land (~9.7 us) — no on-device trig on the ramp critical path,
  and no ACT Sin-table load gating the start of the output stream.
- Pair 1: host ships range-reduced phases r in [-pi,pi] as f16 [64, S]
  (0.5 MB). Mid-stream, a DVE sign-bit clear builds |r| in partitions
  0:64 and one Sin activation per half with per-partition (scale, bias)
  = (-1, pi/2)/(+1, 0) yields [cos^T; sin^T] (arguments inside the
  accurate [-pi/2, pi/2] spline range). This prep rides in pair-0's
  q-loop slack; a dummy 1-column sin preloads the ACT tables during the
  ramp so no mid-stream table stall occurs.
- One K=128 f16 matmul per [128 q x 512 k] output block. PSUM is carved
  into four [128, 1024] half-tiles (2 banks each): per q-tile, psA
  holds k-blocks 0-1 and psB k-blocks 2-3, so VectorE (psA) and ACT
  (psB) recycle PSUM independently; the PSUM chain (matmul + one
  half-evac ~2.4 us per 2 tiles) stays under the DMA drain period.
  Evacuation applies the 1/64 scale and converts to f16.
- Output DMA: 2 q-tiles per [128, 2*S] f16 SBUF block, one 1 MB
  sync-ring (HWDGE) DMA with 8 KB contiguous per-partition descriptors
  (DRAM layout [8 blocks, 128, 2*S] per pair; host unpermutes). All
  output DMAs ride the SP ring so ACT compute never delays an issue;
  ot bufs=8 lets evacuation run well ahead of the drain. The first and
  last blocks stream as 4 x 256 KB quarter-DMAs fired per half-evac,
  starting the HBM write stream earlier and shrinking the final drain.
- All input DMAs are queued on the sync ring ahead of the output blocks
  (pair-0 chunks first, then pair-1), so the wire is never given
  non-critical bytes while ramp-critical ones wait, and input drains
  during the window where output isn't ready yet.
"""

import sys

import numpy as np

try:
    import concourse.bacc as bacc
except ImportError:  # fresh interpreter without the axon site path
    for _p in ("/opt/trn_rl_repo", "/root/.axon_site/_ro/trn_rl_repo"):
        if _p not in sys.path:
            sys.path.insert(0, _p)
    import concourse.bacc as bacc

import concourse.mybir as mybir
import concourse.tile as tile
from concourse.bass_utils import run_bass_kernel_spmd

F32 = mybir.dt.float32
F16 = mybir.dt.float16
U16 = mybir.dt.uint16
UV_DT = F16  # matmul operand dtype
OUT_DT = F16  # device-side output dtype (host upcasts to f32)

B, H, S, D = 2, 8, 2048, 64
N_CORES = 8
PAIRS_PER_CORE = (B * H) // N_CORES  # 2
Q_TILE = 128  # output rows per matmul (PSUM partitions)
K_TILE = 512  # output cols per matmul
N_QT = S // Q_TILE  # 16
BLK = 2  # q-tiles per output DMA block (1 MB f16)
N_BLK = N_QT // BLK  # 8
HC = S // 2  # half-row chunk for pair-1 sin
EC = 2 * K_TILE  # evac chunk (one PSUM half-tile)

_NC_CACHE = {}


def build_kernel():
    """Per-core SPMD program. pin0 [2, 128, S] f16: pair-0 ready
    cos/sin operand blocks (tensor 0 = V from k-phases, 1 = U from
    q-phases). pin1 [2, 64, S] f16: pair-1 range-reduced phases.
    Output out [PAIRS, N_BLK, 128, BLK*S] f16: block j holds q-tiles
    BLK*j..BLK*j+BLK-1 side by side."""
    nc = bacc.Bacc("TRN2", target_bir_lowering=False, debug=False)
    pin0 = nc.dram_tensor("pin0", [2, 128, S], F16, kind="ExternalInput")
    pin1 = nc.dram_tensor("pin1", [2, 64, S], F16, kind="ExternalInput")
    out = nc.dram_tensor(
        "out", [PAIRS_PER_CORE, N_BLK, 128, BLK * S], OUT_DT, kind="ExternalOutput"
    )
    SIN = mybir.ActivationFunctionType.Sin

    with tile.TileContext(nc) as tc:
        with (
            tc.tile_pool(name="const", bufs=1) as cpool,
            tc.tile_pool(name="raw", bufs=1) as rawpool,
            tc.tile_pool(name="uv", bufs=2) as uvpool,
            tc.tile_pool(name="ot", bufs=8) as opool,
            tc.tile_pool(name="psum", bufs=2, space="PSUM") as ppool,
        ):
            # Per-partition Sin affine for pair 1: top half cos via
            # sin(pi/2 - |r|), bottom half sin via sin(r).
            bias = cpool.tile([128, 1], F32)
            scale = cpool.tile([128, 1], F32)
            tabw = cpool.tile([128, 1], F32)
            nc.vector.memset(bias[0:64, :], np.pi / 2)
            nc.vector.memset(bias[64:128, :], 0.0)
            nc.vector.memset(scale[0:64, :], -1.0)
            nc.vector.memset(scale[64:128, :], 1.0)

            raw1 = (
                rawpool.tile([128, S], F16, tag="vraw", name="vraw"),
                rawpool.tile([128, S], F16, tag="uraw", name="uraw"),
            )
            uvs = {}
            for p in range(PAIRS_PER_CORE):
                uvs[p] = (
                    uvpool.tile([128, S], UV_DT, tag="v", name="v"),
                    uvpool.tile([128, S], UV_DT, tag="u", name="u"),
                )

            # All inputs on the sync ring, ramp-critical bytes first: the
            # first q-tile needs u cols 0:128 + v k-blocks in order.
            # (Splitting u/v across both rings lands them in parallel and
            # starts the first matmul ~0.8 us earlier, but the first
            # output DMA does not move — the evacuation-side semaphore
            # latency absorbs it — so single-ring is kept for simplicity.)
            K2 = 2 * K_TILE
            nc.sync.dma_start(out=uvs[0][1][:, 0:K_TILE], in_=pin0[1, :, 0:K_TILE])
            nc.sync.dma_start(out=uvs[0][0][:, 0:K_TILE], in_=pin0[0, :, 0:K_TILE])
            nc.sync.dma_start(out=uvs[0][0][:, K_TILE:K2], in_=pin0[0, :, K_TILE:K2])
            nc.sync.dma_start(out=uvs[0][0][:, K2:S], in_=pin0[0, :, K2:S])
            nc.sync.dma_start(out=uvs[0][1][:, K_TILE:S], in_=pin0[1, :, K_TILE:S])
            nc.sync.dma_start(out=raw1[0][64:128, :], in_=pin1[0])
            nc.sync.dma_start(out=raw1[1][64:128, :], in_=pin1[1])

            # Dummy 1-column sin: pulls the ACT Sin-table loads into the
            # ramp (ACT is otherwise idle there) so pair-1's mid-stream
            # sins don't stall on a ~2.6 us table load.
            nc.scalar.activation(tabw[:], bias[:], SIN)

            # Engine wake-ups: the first PSUM evacuation on a cold engine
            # starts ~0.9 us after its matmul's semaphore fires (vs
            # ~0.1-0.3 us warm). A 1-column op gated on the first input
            # chunk's DMA leaves each engine freshly active right before
            # its first real evacuation, shaving the cold-start gap off
            # the first-output-byte chain.
            wakev = cpool.tile([128, 1], F32)
            wakes = cpool.tile([128, 1], F32)
            nc.vector.tensor_scalar_mul(wakev[:], uvs[0][1][:, 0:1], 1.0)
            nc.scalar.mul(wakes[:], uvs[0][0][:, 0:1], 1.0)

            def abs1(t):
                nc.vector.tensor_scalar(
                    raw1[t][0:64, :].bitcast(U16),
                    raw1[t][64:128, :].bitcast(U16),
                    0x7FFF,
                    None,
                    mybir.AluOpType.bitwise_and,
                )

            def sin1(t, h):
                hs = slice(h * HC, (h + 1) * HC)
                nc.scalar.activation(
                    uvs[1][t][:, hs], raw1[t][:, hs], SIN,
                    bias=bias[:], scale=scale[:],
                )

            def q_tile(p, q, ot, col0, dma_quarters, fine=False):
                v, u = uvs[p][0], uvs[p][1]
                us = u[:, q * Q_TILE : (q + 1) * Q_TILE]
                psA = ppool.tile([128, EC], F32, tag="psA", name="psA")
                psB = ppool.tile([128, EC], F32, tag="psB", name="psB")
                for k in range(2):
                    nc.tensor.matmul(
                        psA[:, k * K_TILE : (k + 1) * K_TILE],
                        us,
                        v[:, k * K_TILE : (k + 1) * K_TILE],
                        start=True,
                        stop=True,
                    )
                for k in range(2):
                    nc.tensor.matmul(
                        psB[:, k * K_TILE : (k + 1) * K_TILE],
                        us,
                        v[:, (k + 2) * K_TILE : (k + 3) * K_TILE],
                        start=True,
                        stop=True,
                    )
                if fine:
                    # First tile of the stream: 512-col evac + 128 KB DMA
                    # chunks, each gated only on its own matmul, so the
                    # first output bytes hit the wire right after matmul
                    # k0 instead of after the whole psA half-tile.
                    for k in range(2):
                        ks = slice(col0 + k * K_TILE, col0 + (k + 1) * K_TILE)
                        nc.vector.tensor_scalar_mul(
                            ot[:, ks], psA[:, k * K_TILE : (k + 1) * K_TILE], 1.0 / D
                        )
                        nc.sync.dma_start(out=dma_quarters[ks], in_=ot[:, ks])
                    for k in range(2):
                        ks = slice(col0 + EC + k * K_TILE, col0 + EC + (k + 1) * K_TILE)
                        nc.scalar.mul(
                            ot[:, ks], psB[:, k * K_TILE : (k + 1) * K_TILE], 1.0 / D
                        )
                        nc.sync.dma_start(out=dma_quarters[ks], in_=ot[:, ks])
                    return
                nc.vector.tensor_scalar_mul(ot[:, col0 : col0 + EC], psA[:], 1.0 / D)
                if dma_quarters is not None:
                    nc.sync.dma_start(
                        out=dma_quarters[col0 : col0 + EC], in_=ot[:, col0 : col0 + EC]
                    )
                nc.scalar.mul(ot[:, col0 + EC : col0 + 2 * EC], psB[:], 1.0 / D)
                if dma_quarters is not None:
                    nc.sync.dma_start(
                        out=dma_quarters[col0 + EC : col0 + 2 * EC],
                        in_=ot[:, col0 + EC : col0 + 2 * EC],
                    )

            # Pair-1 prep spread through pair-0's q-loop: abs once the
            # input DMAs have landed, sins one per block, all done before
            # pair-0's last block.
            prep1 = {
                5: lambda: abs1(0),
                7: lambda: abs1(1),
                8: lambda: sin1(0, 0),
                10: lambda: sin1(0, 1),
                12: lambda: sin1(1, 0),
                14: lambda: sin1(1, 1),
            }

            for p in range(PAIRS_PER_CORE):
                for blk in range(N_BLK):
                    ot = opool.tile([128, BLK * S], OUT_DT, tag="ot", name="ot")
                    split = (p == 0 and blk == 0) or (
                        p == PAIRS_PER_CORE - 1 and blk == N_BLK - 1
                    )
                    dq = None
                    if split:
                        class _Q:  # column-sliced DMA target for this block
                            def __getitem__(_s, cols):
                                return out[p, blk, :, cols]
                        dq = _Q()
                    for j in range(BLK):
                        q = blk * BLK + j
                        q_tile(p, q, ot, j * S, dq, fine=(p == 0 and q == 0))
                        if p == 0 and q in prep1:
                            prep1[q]()
                    if not split:
                        nc.sync.dma_start(out=out[p, blk], in_=ot[:])
    nc.compile()
    return nc


def _prep(ph):
    """[16, S, D] phases -> [16, 64, S] f64 range-reduced transposed."""
    pht = ph.astype(np.float64).transpose(0, 2, 1)  # [16, D, S]
    return np.mod(pht + np.pi, 2 * np.pi) - np.pi


def kernel(phases_q, phases_k, _trace=False):
    pq = np.asarray(phases_q, dtype=np.float32).reshape(B * H, S, D)
    pk = np.asarray(phases_k, dtype=np.float32).reshape(B * H, S, D)
    qr = _prep(pq)  # [16, 64, S] f64
    kr = _prep(pk)

    in_maps = []
    for c in range(N_CORES):
        p0, p1 = 2 * c, 2 * c + 1
        pin0 = np.empty((2, 2 * D, S), dtype=np.float16)  # ready cos/sin
        for t, r in ((0, kr[p0]), (1, qr[p0])):
            pin0[t, :D] = np.cos(r)
            pin0[t, D:] = np.sin(r)
        pin1 = np.stack([kr[p1], qr[p1]]).astype(np.float16)  # [2, 64, S]
        in_maps.append(
            {"pin0": np.ascontiguousarray(pin0), "pin1": np.ascontiguousarray(pin1)}
        )

    if "nc" not in _NC_CACHE:
        _NC_CACHE["nc"] = build_kernel()
    nc = _NC_CACHE["nc"]

    res = run_bass_kernel_spmd(
        nc, in_maps, core_ids=list(range(N_CORES)), trace=_trace
    )
    # [16, N_BLK, 128, BLK*S] -> [16, S, S]: block j holds q-tiles
    # (BLK*j+i) in column slices i*S:(i+1)*S.
    full = np.concatenate([r["out"] for r in res.results], axis=0)
    full = full.reshape(B * H, N_BLK, Q_TILE, BLK, S)
    full = full.transpose(0, 1, 3, 2, 4).reshape(B * H, S, S)
    out = full.astype(np.float32).reshape(B, H, S, S)
    if _trace:
        return out, res
    return out
